# revision 33
# baseline (speedup 1.0000x reference)
"""Causal self-attention (B=4, T=2048, C=1024, H=16) on 8 trn2 NeuronCores.

Sharding: batch x head-half (tensor-parallel over heads, W_proj rows
sharded). Core c handles batch b = c//2 and head-half h = c%2 (heads
[8h, 8h+8)). Each core computes Q/K/V only for its 8 heads (no duplicated
K/V projection), runs causal attention for those heads over the full 2048
tokens, and applies the matching 512-row block of W_proj, producing a
PARTIAL output [1024 feats, 2048 toks]. The host gather sums the two
partials of each batch (the "all-reduce after c_proj" of the tensor-
parallel sharding, folded into the host-side unshard) - zero on-device
collectives.

Per-core pipeline (layouts chosen so no transposes are needed):
  A) QKV projection (bf16): Q^T/K^T feature-major [512, 2048] via lhsT=W
     chunk, rhs=x^T chunk; V token-major, stored [128, kt, head, 65] with
     a ones column (row 64 of the P@V output accumulates the softmax
     denominator l for free).
  B) Attention, q-blocks of 256 ascending, k-tiles of 128 in pairs:
     S^T pair = K^T.T @ Q^T with even/odd heads row-packed on the PE
     (partition offsets 0/64 -> concurrent 64-row matmuls into separate
     PSUM banks); ONE exp per kt-pair on ScalarE ([128, 1024] free) into
     bf16 SBUF; causal mask multiply (DVE) on the 2 diagonal k-tiles only,
     with the sub-diagonal tile trimmed to its valid 128-column half;
     P^T @ V accumulated in PSUM (65 rows: 64 head dims + l); r = 1/l
     (DVE) partition-broadcast via a DRAM round-trip on the SP DMA queue.
     The y normalize of chunk m is deferred until after chunk m+1's
     kt-loop so the round-trip latency never blocks the DVE queue head.
  C) Output projection (float32r) per q-block, deferred behind the next
     q-block's first kt-loop so its inputs (including the odd-head y DMA
     to partitions 64-127) are in flight while the PE stays busy.
  Early q-blocks only need the first x chunks' K/V, so phase A token
  chunks and phase B q-blocks interleave: A0 B0 A1 B1 A2 B2 B3 A3 then
  B4..B7, keeping ScalarE fed throughout.

CC_REPEAT env: build `repeat` copies of the pipeline in one NEFF (for
marginal timing).
"""

import os
import sys

import numpy as np

for _p in ("/opt/trn_rl_repo",):
    if os.path.isdir(_p) and _p not in sys.path:
        sys.path.insert(0, _p)

import ml_dtypes

B, T, C, H = 4, 2048, 1024, 16
HD = C // H  # 64
P = 128
CI = C // P  # 8 contraction chunks of the 1024-dim x features
FM = 4  # feature chunks of 128 within this core's 512 QKV features
NKT = T // P  # 16 k-tiles
QB = 256  # q-block
NQB = T // QB  # 8
NCORE = 8
BF16 = ml_dtypes.bfloat16

_CACHE = {}
LAST_RESULTS = None


def _build():
    from contextlib import ExitStack

    import concourse.bass as bass  # noqa: F401
    import concourse.mybir as mybir
    from concourse import bacc, tile

    dt = mybir.dt
    f32, bf16, f32r = dt.float32, dt.bfloat16, dt.float32r
    EXP = mybir.ActivationFunctionType.Exp

    repeat = int(os.environ.get("CC_REPEAT", "1"))
    nc = bacc.Bacc(
        "TRN2",
        target_bir_lowering=False,
        debug=False,
        enable_asserts=False,
        num_devices=NCORE,
    )
    xt = nc.dram_tensor("xt", [C, T], bf16, kind="ExternalInput").ap()
    wq = nc.dram_tensor("wq", [C, 512], bf16, kind="ExternalInput").ap()
    wk = nc.dram_tensor("wk", [C, 512], bf16, kind="ExternalInput").ap()
    wv = nc.dram_tensor("wv", [C, 512], bf16, kind="ExternalInput").ap()
    wp = nc.dram_tensor("wp", [512, C], f32r, kind="ExternalInput").ap()
    mk = nc.dram_tensor("mk", [P, QB], bf16, kind="ExternalInput").ap()
    out_t = nc.dram_tensor("out_t", [C, T], f32, kind="ExternalOutput").ap()
    out_r = out_t.rearrange("(o p) q -> p o q", p=P)

    with tile.TileContext(nc) as tc, ExitStack() as ctx:
      for _rep in range(repeat):
        rep_ctx = ctx if repeat == 1 else ExitStack()
        res = rep_ctx.enter_context(tc.tile_pool(name="res", bufs=1))
        KT = res.tile([P, FM, T], bf16, name="KT")
        QT = res.tile([P, FM, T], bf16, name="QT")
        V = res.tile([P, NKT, 8, HD + 1], bf16, name="Vt")
        mask = res.tile([P, QB], bf16, name="mask")
        wq_sb = res.tile([P, CI, 512], bf16, name="wq_sb")
        wk_sb = res.tile([P, CI, 512], bf16, name="wk_sb")
        wv_sb = res.tile([P, CI, 512], bf16, name="wv_sb")
        wp_sb = res.tile([P, FM, C], f32r, name="wp_sb")
        ones_sb = res.tile([1, HD], f32r, name="ones_sb")
        ones_f = res.tile([1, HD], f32, name="ones_f")

        bpools = ExitStack()
        xtp = bpools.enter_context(tc.tile_pool(name="xtp", bufs=2))
        psS = bpools.enter_context(tc.tile_pool(name="psS", bufs=2, space="PSUM"))
        psO = bpools.enter_context(tc.tile_pool(name="psO", bufs=2, space="PSUM"))
        pP = bpools.enter_context(tc.tile_pool(name="pP", bufs=6))
        pR = bpools.enter_context(tc.tile_pool(name="pR", bufs=2))
        pRB = bpools.enter_context(tc.tile_pool(name="pRB", bufs=2))
        pY = bpools.enter_context(tc.tile_pool(name="pY", bufs=8))
        pYS = bpools.enter_context(tc.tile_pool(name="pYS", bufs=2))
        pOS = bpools.enter_context(tc.tile_pool(name="pOS", bufs=2))
        pRD = bpools.enter_context(tc.tile_pool(name="pRD", bufs=4, space="DRAM"))
        stA = ExitStack()
        psA = stA.enter_context(tc.tile_pool(name="psA", bufs=2, space="PSUM"))

        # DMA order matters: the first Q matmul needs only wq's first m-chunk
        # + xt chunk 0, so those go first; wk/wv arrive during Q(th0), the
        # mask by B0's diagonal, wp (phase C weights) after xt chunk 1.
        wq_r = wq.rearrange("(o p) f -> p o f", p=P)
        nc.sync.dma_start(wq_sb[:, :, 0:P], wq_r[:, :, 0:P])

        xt_tiles = {}

        def a_dma(th):
            xt_sb = xtp.tile([P, CI, 512], bf16, name="xt_sb")
            xt_tiles[th] = xt_sb
            nc.sync.dma_start(
                xt_sb,
                xt[:, th * 512:(th + 1) * 512].rearrange("(o p) t -> p o t", p=P),
            )
            if th == 0:
                nc.sync.dma_start(wq_sb[:, :, P:], wq_r[:, :, P:])
                nc.sync.dma_start(wk_sb, wk.rearrange("(o p) f -> p o f", p=P))
                nc.sync.dma_start(wv_sb, wv.rearrange("(o p) f -> p o f", p=P))
                nc.sync.dma_start(mask, mk)
                nc.gpsimd.memset(V[:, :, :, HD:], 1.0)
                nc.gpsimd.memset(ones_f, 1.0)
                with nc.allow_low_precision(reason="f32r is full-width f32 storage"):
                    nc.vector.tensor_copy(ones_sb, ones_f)
            if th == 1:
                nc.sync.dma_start(wp_sb, wp.rearrange("(o p) f -> p o f", p=P))

        def a_group(th, i):
            """One projection group: i 0-3 Q(m=i), 4-7 K(m=i-4), 8-11 V(kt4=i-8)."""
            xt_sb = xt_tiles[th]
            ps = psA.tile([P, 512], f32, name="psA_t")
            if i < 8:
                dest, w_sb, m = (QT, wq_sb, i) if i < 4 else (KT, wk_sb, i - 4)
                for ci in range(CI):
                    nc.tensor.matmul(
                        ps,
                        lhsT=w_sb[:, ci, m * P:(m + 1) * P],
                        rhs=xt_sb[:, ci],
                        start=(ci == 0),
                        stop=(ci == CI - 1),
                    )
                nc.vector.tensor_copy(dest[:, m, th * 512:(th + 1) * 512], ps)
            else:
                kt4 = i - 8
                kt = th * 4 + kt4
                for ci in range(CI):
                    nc.tensor.matmul(
                        ps,
                        lhsT=xt_sb[:, ci, kt4 * P:(kt4 + 1) * P],
                        rhs=wv_sb[:, ci],
                        start=(ci == 0),
                        stop=(ci == CI - 1),
                    )
                nc.vector.tensor_copy(
                    V[:, kt, :, :HD], ps.rearrange("p (h d) -> p h d", d=HD)
                )

        # ---- Phase B/C machinery ----
        state = {
            "pend_y": None,   # (o_ps, y_qb, ystage, m, rb) awaiting normalize
            "pend_y3": None,  # same, for m==3 (flushed at next qb's start)
            "pend_c": [],     # q-blocks whose C phase is not yet emitted
            "psC": None,
            "yodd": None,     # (y_qb, ystage) awaiting the odd-half DMA
        }

        def emit_ynorm(ent):
            o_ps, y_qb, ystage, m, rb_sb = ent
            nc.vector.tensor_mul(y_qb[0:HD, m, :], o_ps[0:HD, 0:QB], rb_sb[:, 0:QB])
            nc.vector.tensor_mul(ystage[:, m, :], o_ps[0:HD, QB:], rb_sb[:, QB:])

        def emit_yodd():
            y_qb, ystage = state["yodd"]
            nc.sync.dma_start(y_qb[HD:P, :, :].bitcast(f32), ystage)
            state["yodd"] = None

        def emit_C(qb, y_qb):
            q0 = qb * QB
            osb = pOS.tile([P, CI, QB], f32, name="osb")
            for co in range(CI):
                ps = state["psC"].tile([P, QB], f32, name="psC_t")
                for ci in range(FM):
                    nc.tensor.matmul(
                        ps,
                        lhsT=wp_sb[:, ci, co * P:(co + 1) * P],
                        rhs=y_qb[:, ci, :],
                        start=(ci == 0),
                        stop=(ci == FM - 1),
                    )
                nc.vector.tensor_copy(osb[:, co, :], ps)
            nc.sync.dma_start(out_r[:, :, q0:q0 + QB], osb)

        def gen_B():
            """Generator over phase B; yields the current q-block before each
            PE quantum (kt-pair / diagonal) so the scheduler can interleave
            phase-A projection groups."""
            for qb in range(NQB):
              q0 = qb * QB
              y_qb = pY.tile([P, FM, QB], f32r, name="y_qb")
              ystage = pYS.tile([HD, FM, QB], f32, name="ystage")
              for m in range(FM):
                if m == 0 and state["pend_y3"] is not None:
                    emit_ynorm(state["pend_y3"])
                    state["pend_y3"] = None
                    emit_yodd()
                o_ps = psO.tile([P, 2 * QB], f32, name="o_ps")

                def s_mm(s, j, kt, qoff, ncol):
                    # Each hh half of the s tile is one PSUM bank; start=True
                    # on j==0 pends the whole bank (j==1 overwrites its
                    # pending half), stop closes the bank's group on j==1.
                    for hh in range(2):
                        hp = hh * HD
                        nc.tensor.matmul(
                            s[:, hh, j, 0:ncol],
                            lhsT=KT[hp:hp + HD, m, kt * P:(kt + 1) * P],
                            rhs=QT[hp:hp + HD, m, q0 + qoff:q0 + qoff + ncol],
                            start=(j == 0),
                            stop=(j == 1),
                        )

                def pv(pt, j, kt, hh, c0, ncol, start, stop, skip=False):
                    nc.tensor.matmul(
                        o_ps[0:HD + 1, hh * QB + c0:hh * QB + c0 + ncol],
                        lhsT=V[:, kt, 2 * m + hh, :],
                        rhs=pt[:, hh, j, 0:ncol],
                        start=start,
                        stop=stop,
                        skip_group_check=skip,
                    )

                # The o tile is ONE PSUM bank holding both heads: exactly one
                # start (pends the whole bank) and one stop across the whole
                # kt loop.
                for ktp in range(qb):  # full kt pairs
                    yield qb
                    s = psS.tile([P, 2, 2, QB], f32, name="s_t")
                    pt = pP.tile([P, 2, 2, QB], bf16, name="p_t")
                    for j in range(2):
                        s_mm(s, j, 2 * ktp + j, 0, QB)
                    nc.scalar.activation(pt, s, EXP, scale=0.125)
                    for j in range(2):
                        for hh in range(2):
                            pv(pt, j, 2 * ktp + j, hh, 0, QB,
                               start=(ktp == 0 and j == 0 and hh == 0),
                               stop=False)

                # diagonal pair: kts (2qb, 2qb+1); d1 trimmed to the valid
                # upper half of the q-block.
                yield qb
                s = psS.tile([P, 2, 2, QB], f32, name="s_t")
                pt = pP.tile([P, 2, 2, QB], bf16, name="p_t")
                s_mm(s, 0, 2 * qb, 0, QB)
                s_mm(s, 1, 2 * qb + 1, QB // 2, QB // 2)
                nc.scalar.activation(pt[:, :, 0, :], s[:, :, 0, :], EXP, scale=0.125)
                nc.scalar.activation(
                    pt[:, :, 1, 0:QB // 2], s[:, :, 1, 0:QB // 2], EXP, scale=0.125
                )
                for hh in range(2):
                    nc.vector.tensor_mul(pt[:, hh, 0, :], pt[:, hh, 0, :], mask)
                    nc.vector.tensor_mul(
                        pt[:, hh, 1, 0:QB // 2], pt[:, hh, 1, 0:QB // 2],
                        mask[:, 0:QB // 2],
                    )
                if qb == 0:
                    for hh in range(2):
                        pv(pt, 0, 2 * qb, hh, 0, QB, start=(hh == 0), stop=False)
                    for hh in range(2):
                        pv(pt, 1, 2 * qb + 1, hh, QB // 2, QB // 2,
                           start=False, stop=(hh == 1))
                else:
                    for hh in range(2):
                        pv(pt, 1, 2 * qb + 1, hh, QB // 2, QB // 2,
                           start=False, stop=False)
                    for hh in range(2):
                        pv(pt, 0, 2 * qb, hh, 0, QB, start=False, stop=(hh == 1))

                # r = 1/l; broadcast to 64 partitions via DRAM round-trip
                # (SP queue). The very last chunk instead broadcasts with a
                # rank-1 PE matmul so the tail avoids the DMA latency.
                if qb == NQB - 1 and m == FM - 1:
                    r_sb = pR.tile([1, 2 * QB], f32r, name="r_last")
                    with nc.allow_low_precision(reason="f32r is full-width f32 storage"):
                        nc.vector.reciprocal(r_sb, o_ps[HD:HD + 1, :])
                    state["r_last"] = r_sb
                    rb_sb = None
                else:
                    r_sb = pR.tile([1, 2 * QB], f32, name="r_sb")
                    nc.vector.reciprocal(r_sb, o_ps[HD:HD + 1, :])
                    r_dr = pRD.tile([1, 2 * QB], f32, name="r_dr")
                    nc.sync.dma_start(r_dr, r_sb)
                    rb_sb = pRB.tile([HD, 2 * QB], f32, name="rb_sb")
                    nc.sync.dma_start(rb_sb, r_dr.to_broadcast((HD, 2 * QB)))

                ent = (o_ps, y_qb, ystage, m, rb_sb)
                if m >= 1:
                    emit_ynorm(state["pend_y"])
                if state["pend_c"] and state["psC"] is not None:
                    emit_C(*state["pend_c"].pop(0))
                state["pend_y"] = ent
              state["pend_y3"] = state["pend_y"]
              state["pend_y"] = None
              state["yodd"] = (y_qb, ystage)
              state["pend_c"].append((qb, y_qb))

        # ---- interleaved emission schedule ----
        # Drip phase-A projection groups between phase-B PE quanta: one group
        # per 2 quanta keeps ScalarE continuously fed while A streams; the
        # forced-finish loop guarantees the K/V tiles a q-block needs exist
        # before its first S matmul.
        from collections import deque

        stC = ExitStack()
        a_left = deque((th, i) for th in range(4) for i in range(12))
        dma_done = set()

        def ensure_dma(th):
            if th < 4 and th not in dma_done:
                a_dma(th)
                dma_done.add(th)

        def emit_next_a():
            th, i = a_left.popleft()
            ensure_dma(th)
            a_group(th, i)
            if i == 8:  # prefetch next chunk's x while 3 groups remain
                ensure_dma(th + 1)

        gb = gen_B()
        b_cnt = 0
        for qb_cur in gb:
            need_th = (2 * qb_cur + 1) // 4
            while a_left and a_left[0][0] <= need_th:
                emit_next_a()
            if not a_left and state["psC"] is None:
                stA.close()
                state["psC"] = stC.enter_context(
                    tc.tile_pool(name="psC", bufs=2, space="PSUM")
                )
            b_cnt += 1
            if b_cnt % 2 == 0 and a_left:
                emit_next_a()
        while a_left:
            emit_next_a()
        if state["psC"] is None:
            stA.close()
            state["psC"] = stC.enter_context(
                tc.tile_pool(name="psC", bufs=2, space="PSUM")
            )
        rb_t = psO.tile([P, 2 * QB], f32, name="o_ps")
        rb_ps = rb_t[0:HD, :]
        nc.tensor.matmul(
            rb_ps,
            lhsT=ones_sb,
            rhs=state["r_last"],
            start=True,
            stop=True,
        )
        rb_last = pRB.tile([HD, 2 * QB], f32, name="rb_sb")
        nc.vector.tensor_copy(rb_last, rb_ps)
        o_ps, y_qb, ystage, m, _ = state["pend_y3"]
        emit_ynorm((o_ps, y_qb, ystage, m, rb_last))
        state["pend_y3"] = None
        emit_yodd()
        while state["pend_c"]:
            emit_C(*state["pend_c"].pop(0))
        stC.close()
        bpools.close()

        if repeat != 1:
            rep_ctx.close()

    nc.compile()
    return nc


def _prep_inputs(x, W_attn, W_proj):
    """Host-side shard/layout prep. Pure data movement + dtype casts."""
    x = np.asarray(x, dtype=np.float32)
    W_attn = np.asarray(W_attn, dtype=np.float32)
    W_proj = np.asarray(W_proj, dtype=np.float32)

    # Causal mask tile [k, q] for a 128-row k-tile aligned with the q-block
    # start: valid iff k <= q.
    kk = np.arange(P)[:, None]
    qq = np.arange(QB)[None, :]
    mask = (kk <= qq).astype(np.float32).astype(BF16)

    xt_b = [np.ascontiguousarray(x[b].T).astype(BF16) for b in range(B)]
    wq_h = [np.ascontiguousarray(W_attn[:, h * 512:(h + 1) * 512]).astype(BF16)
            for h in range(2)]
    wk_h = [np.ascontiguousarray(W_attn[:, C + h * 512:C + (h + 1) * 512]).astype(BF16)
            for h in range(2)]
    wv_h = [np.ascontiguousarray(W_attn[:, 2 * C + h * 512:2 * C + (h + 1) * 512]).astype(BF16)
            for h in range(2)]
    wp_h = [np.ascontiguousarray(W_proj[h * 512:(h + 1) * 512, :]) for h in range(2)]

    in_maps = []
    for c in range(NCORE):
        b, h = c // 2, c % 2
        in_maps.append(
            {
                "xt": xt_b[b],
                "wq": wq_h[h],
                "wk": wk_h[h],
                "wv": wv_h[h],
                "wp": wp_h[h],
                "mk": mask,
            }
        )
    return in_maps


def kernel(x, W_attn, W_proj):
    global LAST_RESULTS
    from concourse.bass_utils import run_bass_kernel_spmd

    if "nc" not in _CACHE:
        _CACHE["nc"] = _build()
    nc = _CACHE["nc"]

    in_maps = _prep_inputs(x, W_attn, W_proj)
    trace = os.environ.get("CC_TRACE", "0") == "1"
    res = run_bass_kernel_spmd(nc, in_maps, core_ids=list(range(NCORE)), trace=trace)
    LAST_RESULTS = res

    out = np.empty((B, T, C), dtype=np.float32)
    for b in range(B):
        part = res.results[2 * b]["out_t"] + res.results[2 * b + 1]["out_t"]
        out[b] = part.T
    return out
